# revision 1
# baseline (speedup 1.0000x reference)
"""Bass/Trainium2 kernel for BailingAttention (GQA prefill, causal, RoPE).

Sharding: tensor-parallel over heads across 8 NeuronCores. Each core computes
2 query heads + its group's shared KV head end-to-end (QKV projection, RoPE,
causal attention, output projection) and writes a partial [T, HID] output;
the host sums the 8 partials (the row-parallel all-reduce).

Layouts on device (partition dim first):
  hiddenT  [HID, T]   (host-transposed)  -> moving operand of QKV matmuls
  qT/kT    [D, T]     per head           -> RoPE applied in this layout
  v        [T, D]     natural            -> PV stationary (via PE transpose)
  scoresT  [kt, qt]   exp'd on ACT; denominator accumulated on PE via an
                      all-ones stationary (replicated column sums in PSUM)
  ctxT     [D, T]     -> stationary of the output projection

All matmuls run in fp32r (TF32-like: fp32 RNE-rounded to 11 mantissa bits)
at full PE speed. DRAM-sourced fp32r operands are pre-rounded bit-exactly on
the host so plain HWDGE DMAs suffice; on-device producers write fp32r
directly (the cast rounds).

The output projection for a 512-token block is emitted right after that
block's attention so its PSUM-evict copies and 1 MB output DMAs overlap the
next block's attention instead of running exposed at the end.
"""

import numpy as np

import concourse.bass as bass
import concourse.mybir as mybir
import concourse.tile as tile
from concourse import bacc, bass_utils
from concourse.bass import ts

F32 = mybir.dt.float32
F32R = mybir.dt.float32r
AF = mybir.ActivationFunctionType
OP = mybir.AluOpType

H, KV, D, HID, T = 16, 4, 128, 2048, 2048
THETA = 10000.0
N_CORES = 8
QH = H // N_CORES            # query heads per core = 2
TB = 512                     # token block (matmul moving N)
NTB = T // TB                # 4
HCN = HID // 128             # 16 h-chunks
NKT_TILES = T // 128         # 16 key tiles
SCALE = float(D) ** -0.5
PIPE = 3                     # attention software-pipeline depth (score MMs ahead)


def _to_f32r(a: np.ndarray) -> np.ndarray:
    """Round fp32 to fp32r bits (RNE to 11-bit mantissa) — bit-exactly what
    the hardware cast produces, so raw HWDGE DMA into f32r tiles is lossless."""
    b = np.ascontiguousarray(a, np.float32).view(np.uint32).astype(np.uint64)
    r = ((b + 0x7FF + ((b >> 12) & 1)) & 0xFFFFF000).astype(np.uint32)
    return r.view(np.float32)


def _build():
    nc = bacc.Bacc("TRN2", target_bir_lowering=False, debug=False,
                   num_devices=N_CORES)

    hT_d = nc.dram_tensor("hiddenT", [HID, T], F32R, kind="ExternalInput").ap()
    w_d = nc.dram_tensor("w_local", [HID, 4 * 128], F32R, kind="ExternalInput").ap()
    wo_d = nc.dram_tensor("wo_local", [2 * 128, HID], F32R, kind="ExternalInput").ap()
    cos_d = nc.dram_tensor("cosT", [128, T], F32, kind="ExternalInput").ap()
    sin_d = nc.dram_tensor("sinT", [128, T], F32, kind="ExternalInput").ap()
    mask_d = nc.dram_tensor("masks", [128, 4 * TB], F32, kind="ExternalInput").ap()
    ones_d = nc.dram_tensor("ones", [128, 128], F32R, kind="ExternalInput").ap()
    id_d = nc.dram_tensor("ident", [128, 128], F32, kind="ExternalInput").ap()
    out_d = nc.dram_tensor("out_partial", [T, HID], F32, kind="ExternalOutput").ap()

    with tile.TileContext(nc) as tc:
        with tc.tile_pool(name="const", bufs=1) as cpool, \
             tc.tile_pool(name="acts", bufs=1) as apool:
            # Resident constants. DMA emission order is load-bearing: the
            # QKV stream needs w-chunks + hT tiles first; everything else is
            # deferred so it doesn't delay the first matmuls.
            w_sb = cpool.tile([128, HCN, 512], F32R)
            wo_sb = cpool.tile([128, 2, HID], F32R)
            cos_sb = cpool.tile([128, T], F32)
            sin_sb = cpool.tile([128, T], F32)
            mask_sb = cpool.tile([128, 4, TB], F32)
            ones_sb = cpool.tile([128, 128], F32R)
            id_sb = cpool.tile([128, 128], F32)

            w_view = w_d.rearrange("(hc p) n -> hc p n", p=128)

            # persistent per-core activations
            qrT = [apool.tile([128, T], F32R, name=f"qrT{i}", tag=f"qrT{i}")
                   for i in range(QH)]
            krT = apool.tile([128, T], F32R)
            v_nat = apool.tile([128, NKT_TILES, 128], F32R)
            ctxT = [apool.tile([128, T], F32R, name=f"ctxT{i}", tag=f"ctxT{i}")
                    for i in range(QH)]

            hT_view = hT_d.rearrange("(hc p) t -> hc p t", p=128)

            # ================= Phase 1: QKV projection (+RoPE, v transpose) ==
            with tc.tile_pool(name="hstream", bufs=8) as hpool, \
                 tc.tile_pool(name="p1tmp", bufs=3) as tpool, \
                 tc.tile_pool(name="p1psum", bufs=1, space="PSUM") as qkv_ps_pool, \
                 tc.tile_pool(name="p1psumv", bufs=2, space="PSUM") as vps_pool:
                for b in range(NTB):
                    ps_qkv = [qkv_ps_pool.tile([128, TB], F32, name=f"psqkv{n}",
                                               tag=f"qkv{n}") for n in range(4)]
                    for hc in range(HCN):
                        if b == 0:
                            nc.sync.dma_start(w_sb[:, hc, :], w_view[hc])
                        hT_t = hpool.tile([128, TB], F32R)
                        nc.sync.dma_start(hT_t[:], hT_view[hc, :, ts(b, TB)])
                        for n in range(4):
                            nc.tensor.matmul(ps_qkv[n][:], w_sb[:, hc, ts(n, 128)],
                                             hT_t[:], start=(hc == 0),
                                             stop=(hc == HCN - 1))
                    if b == 0:
                        nc.scalar.dma_start(id_sb[:], id_d)
                        nc.scalar.dma_start(ones_sb[:], ones_d)
                    nc.scalar.dma_start(cos_sb[:, ts(b, TB)], cos_d[:, ts(b, TB)])
                    nc.scalar.dma_start(sin_sb[:, ts(b, TB)], sin_d[:, ts(b, TB)])
                    # Evict all four accumulators first (frees PSUM for the
                    # next block's matmuls), then RoPE / v-transpose.
                    x_sbs = []
                    for n in range(4):
                        x_sb = tpool.tile([128, TB], F32, tag=f"ropex{n}",
                                          name=f"x_sb{n}")
                        if n % 2 == 0:
                            nc.scalar.copy(x_sb[:], ps_qkv[n][:])
                        else:
                            nc.vector.tensor_copy(x_sb[:], ps_qkv[n][:])
                        x_sbs.append(x_sb)
                    for n in range(3):
                        dst = qrT[n] if n < QH else krT
                        x_sb = x_sbs[n]
                        xsw = tpool.tile([128, TB], F32, tag="ropesw")
                        nc.scalar.dma_start(xsw[0:64, :], x_sb[64:128, :])
                        nc.scalar.dma_start(xsw[64:128, :], x_sb[0:64, :])
                        t2 = tpool.tile([128, TB], F32, tag="ropet2")
                        nc.gpsimd.tensor_tensor(out=t2[:], in0=xsw[:],
                                                in1=sin_sb[:, ts(b, TB)], op=OP.mult)
                        m1 = tpool.tile([128, TB], F32, tag="ropem1")
                        nc.gpsimd.tensor_tensor(out=m1[:], in0=x_sb[:],
                                                in1=cos_sb[:, ts(b, TB)], op=OP.mult)
                        nc.vector.tensor_tensor(out=dst[:, ts(b, TB)], in0=m1[:],
                                                in1=t2[:], op=OP.add)
                    vT_sb = x_sbs[3]
                    for j in range(4):
                        ps_v = vps_pool.tile([128, 128], F32)
                        nc.tensor.transpose(ps_v[:], vT_sb[:, ts(j, 128)], id_sb[:])
                        nc.vector.tensor_copy(v_nat[:, 4 * b + j, :], ps_v[:])
                    if b == 2:
                        nc.scalar.dma_start(
                            mask_sb[:], mask_d.rearrange("p (m n) -> p m n", n=TB))
                        nc.scalar.dma_start(
                            wo_sb[:], wo_d.rearrange("(c p) n -> p c n", p=128))

            # ============ Phase 2+3: causal attention + output projection ====
            # Per 512-token block: attention for both heads, then that block's
            # output projection (its copies/DMAs overlap the next block).
            with tc.tile_pool(name="p2exp", bufs=PIPE + 5) as epool, \
                 tc.tile_pool(name="p2tmp", bufs=2) as t2pool, \
                 tc.tile_pool(name="p3out", bufs=2) as opool, \
                 tc.tile_pool(name="p2ps_s", bufs=PIPE + 1, space="PSUM") as sps_pool, \
                 tc.tile_pool(name="p2ps_c", bufs=1, space="PSUM") as cps_pool, \
                 tc.tile_pool(name="p2ps_d", bufs=1, space="PSUM") as dps_pool, \
                 tc.tile_pool(name="p3psum", bufs=2, space="PSUM") as ops_pool:
                for b in range(NTB):
                    nkt = 4 * (b + 1)
                    for qh in range(QH):
                        ctx_ps = cps_pool.tile([128, TB], F32, name="ctx_ps")
                        den_ps = dps_pool.tile([128, TB], F32, name="den_ps")
                        e_tiles = [None] * nkt

                        def emit_score(kt, b=b, qh=qh, e_tiles=e_tiles):
                            s_ps = sps_pool.tile([128, TB], F32, name="s_ps")
                            nc.tensor.matmul(s_ps[:], krT[:, ts(kt, 128)],
                                             qrT[qh][:, ts(b, TB)],
                                             start=True, stop=True)
                            e_sb = epool.tile([128, TB], F32R, name="e_sb",
                                              tag="exp")
                            nc.scalar.activation(e_sb[:], s_ps[:], AF.Exp,
                                                 scale=SCALE)
                            if kt >= 4 * b:   # diagonal tile: causal mask
                                nc.vector.tensor_tensor(
                                    out=e_sb[:], in0=e_sb[:],
                                    in1=mask_sb[:, kt - 4 * b, :], op=OP.mult)
                            e_tiles[kt] = e_sb

                        def emit_consume(kt, nkt=nkt, ctx_ps=ctx_ps,
                                         den_ps=den_ps, e_tiles=e_tiles):
                            e_sb = e_tiles[kt]
                            nc.tensor.matmul(ctx_ps[:], v_nat[:, kt, :], e_sb[:],
                                             start=(kt == 0), stop=(kt == nkt - 1))
                            nc.tensor.matmul(den_ps[:], ones_sb[:], e_sb[:],
                                             start=(kt == 0), stop=(kt == nkt - 1))

                        # Head 1's consumes start deeper so its score MMs
                        # cover head 0's recip/normalize chain (the single
                        # ctx PSUM bank frees only after that chain).
                        depth = PIPE if qh == 0 else min(PIPE + 2, nkt)
                        for kt in range(nkt + depth):
                            if kt < nkt:
                                emit_score(kt)
                            if kt >= depth:
                                emit_consume(kt - depth)

                        recip = t2pool.tile([128, TB], F32, tag="recip",
                                            name="recip")
                        nc.vector.reciprocal(recip[:], den_ps[:])
                        nc.vector.tensor_tensor(out=ctxT[qh][:, ts(b, TB)],
                                                in0=ctx_ps[:], in1=recip[:],
                                                op=OP.mult)

                    # ---- output projection for this block's 4 token tiles ----
                    for tt in range(4 * b, 4 * b + 4):
                        o_sb = opool.tile([128, HID], F32, name="o_sb")
                        for n in range(4):
                            ps_o = ops_pool.tile([128, 512], F32, name="ps_o")
                            for qh in range(QH):
                                nc.tensor.matmul(ps_o[:], ctxT[qh][:, ts(tt, 128)],
                                                 wo_sb[:, qh, ts(n, 512)],
                                                 start=(qh == 0),
                                                 stop=(qh == QH - 1))
                            if n == 0:
                                nc.scalar.copy(o_sb[:, ts(n, 512)], ps_o[:])
                            else:
                                nc.vector.tensor_copy(o_sb[:, ts(n, 512)], ps_o[:])
                        nc.sync.dma_start(out_d[ts(tt, 128), :], o_sb[:])

    nc.compile()
    return nc


_NC_CACHE = None


def _get_nc():
    global _NC_CACHE
    if _NC_CACHE is None:
        _NC_CACHE = _build()
    return _NC_CACHE


def _host_tables(position_ids: np.ndarray):
    pos = np.asarray(position_ids, np.float32)
    inv_freq = (1.0 / (THETA ** (np.arange(0, D, 2, dtype=np.float32) / D)))
    ang = pos[:, None] * inv_freq[None, :]          # [T, 64] f32
    cos = np.cos(ang).T.astype(np.float32)          # [64, T]
    sin = np.sin(ang).T.astype(np.float32)
    cosT = np.concatenate([cos, cos], axis=0)       # [128, T]
    sinT = np.concatenate([-sin, sin], axis=0)
    return cosT, sinT


def _host_masks():
    r = np.arange(128)[:, None]
    c = np.arange(TB)[None, :]
    m = [(c - r - 128 * i >= 0).astype(np.float32) for i in range(4)]
    return np.concatenate(m, axis=1)                # [128, 4*TB]


def kernel(hidden_states, position_ids, Wqkv, Wo):
    hidden_states = np.asarray(hidden_states, np.float32)
    Wqkv = np.asarray(Wqkv, np.float32)
    Wo = np.asarray(Wo, np.float32)

    nc = _get_nc()

    hiddenT = _to_f32r(hidden_states.T)
    cosT, sinT = _host_tables(position_ids)
    masks = _host_masks()
    ones = np.ones((128, 128), np.float32)
    ident = np.eye(128, dtype=np.float32)

    wq = Wqkv[:, : H * D]
    wk = Wqkv[:, H * D: (H + KV) * D]
    wv = Wqkv[:, (H + KV) * D:]

    in_maps = []
    for c in range(N_CORES):
        kvh = (c * QH) // (H // KV)
        w_local = np.concatenate(
            [wq[:, (c * QH) * D: (c * QH + 1) * D],
             wq[:, (c * QH + 1) * D: (c * QH + 2) * D],
             wk[:, kvh * D: (kvh + 1) * D],
             wv[:, kvh * D: (kvh + 1) * D]], axis=1)
        wo_local = Wo[c * QH * D: (c + 1) * QH * D, :]
        in_maps.append({
            "hiddenT": hiddenT,
            "w_local": _to_f32r(w_local),
            "wo_local": _to_f32r(wo_local),
            "cosT": cosT, "sinT": sinT, "masks": masks,
            "ones": ones, "ident": ident,
        })

    res = bass_utils.run_bass_kernel_spmd(nc, in_maps,
                                          core_ids=list(range(N_CORES)))
    parts = np.stack([res.results[c]["out_partial"] for c in range(N_CORES)], 0)
    return parts.sum(axis=0, dtype=np.float32)



# revision 28
# speedup vs baseline: 1.3448x; 1.3448x over previous
"""Bass/Trainium2 kernel for BailingAttention (GQA prefill, causal, RoPE).

Sharding: tensor-parallel over heads across 8 NeuronCores. Each core computes
2 query heads + its group's shared KV head end-to-end and writes a partial
[T, HID] output (bf16); the host sums the 8 partials (row-parallel
all-reduce) and applies the global dequant scale.

Numerics (rel tolerance 2e-2; this lands ~1e-2):
  - QKV and output projections are 3-term split-fp8 DoubleRow matmuls:
    x ~ hi + res, both e4m3 in SHARED scale units, so hi@hi + hi@res +
    res@hi accumulates in one PSUM group at ~0.1% error and 3/4 the fp32r
    PE cost (DoubleRow = 0.5 cyc/row over a 2x128 contraction).
  - Scores stay fp32r (the exp is error-sensitive).
  - exp outputs (e) are e4m3; PV and the softmax denominator run DoubleRow
    over adjacent key-tile PAIRS (e pair tiles [128,2,512]); v is hi+res.
  - All scales are powers of two, folded into host tables (cos/sin carry
    the QKV dequant, the denominator 'ones' stationary carries the ctx
    scale, the final dequant happens host-side during the partial sum).

Schedule: one software-pipelined emission stream. Per 512-token block b:
pass A (q heads) QKV matmuls interleaved with attention(b-1) leftovers and
out-proj(b-1); RoPE(q); pass B (k/v) interleaved with attention(b) head 0's
non-diagonal work. One attention head is in flight at a time so PSUM fits:
QKV ring 2 banks + score-pair ring 4 + ctx 1 + den 1 = 8.

Engine discipline (each engine's FIFO order gates its consumers):
  ACT  = exps only, plus issue-only DMAs (xsw swaps, v DMA-transpose).
  DVE  = psum evicts (q0/k/v), RoPE m1+add, softmax recip/c32/ctx_hi,
         half the out-proj evicts.
  Pool = q1 evict, RoPE t2, causal tri-masks+memsets (shrunk to [128,128]),
         ctx_re, v quantize.
  SP   = all dependency-free input DMAs + output DMAs (emitted late).
Diagonal score pairs are exact-causal: the moving range shrinks to >=256
columns, exp shrinks with it, masks act on [128,128] triangles only.
"""

import numpy as np
import ml_dtypes

import concourse.bass as bass
import concourse.mybir as mybir
import concourse.tile as tile
from concourse import bacc, bass_utils
from concourse.bass import ts

F32 = mybir.dt.float32
F32R = mybir.dt.float32r
F8 = mybir.dt.float8e4
BF16 = mybir.dt.bfloat16
AF = mybir.ActivationFunctionType
OP = mybir.AluOpType
DR = mybir.MatmulPerfMode.DoubleRow
NPF8 = ml_dtypes.float8_e4m3
NPBF = ml_dtypes.bfloat16

H, KV, D, HID, T = 16, 4, 128, 2048, 2048
THETA = 10000.0
N_CORES = 8
QH = H // N_CORES            # query heads per core = 2
TB = 512                     # token block (matmul moving N)
NTB = T // TB                # 4
NPAIR = HID // 256           # 8 contraction pairs for QKV
SCALE = float(D) ** -0.5

S_H = 32.0                   # hidden quant scale
S_W = 1024.0                 # Wqkv quant scale
S_V = 4.0                    # on-device v scale (v tiles = 4*v_true)
S_ADJ = S_V / (S_H * S_W)    # v psum -> scaled-v evict factor
ALPHA = 0.125                # ones value: ctx_hi = (S_V/ALPHA)*ctx = 32*ctx
S_WO = 1024.0                # Wo quant scale
S_OUT = (S_V / ALPHA) * S_WO  # host-side dequant of the partial outputs


def _interleave(main, filler):
    """Emit `main` and `filler` unit lists proportionally merged."""
    if not filler or not main:
        for u in main + filler:
            u()
        return
    r = len(main) / len(filler)
    fi = 0
    acc = 0.0
    for u in main:
        u()
        acc += 1.0
        while fi < len(filler) and acc >= r:
            filler[fi]()
            fi += 1
            acc -= r
    while fi < len(filler):
        filler[fi]()
        fi += 1


def _build():
    nc = bacc.Bacc("TRN2", target_bir_lowering=False, debug=False,
                   num_devices=N_CORES)

    h_hi_d = nc.dram_tensor("h_hi", [HID, T], F8, kind="ExternalInput").ap()
    h_re_d = nc.dram_tensor("h_re", [HID, T], F8, kind="ExternalInput").ap()
    # w split by output-column pair: A = n in (0,1) [q heads], B = n in (2,3)
    w_d = {}
    for hv in ("hi", "re"):
        for ab in ("A", "B"):
            w_d[(hv, ab)] = nc.dram_tensor(
                f"w_{hv}{ab}", [128, NPAIR, 2, 2, 128], F8,
                kind="ExternalInput").ap()
    wo_hi_d = nc.dram_tensor("wo_hi", [128, 2, HID], F8,
                             kind="ExternalInput").ap()
    wo_re_d = nc.dram_tensor("wo_re", [128, 2, HID], F8,
                             kind="ExternalInput").ap()
    cos_d = nc.dram_tensor("cosT", [128, T], BF16, kind="ExternalInput").ap()
    sin_d = nc.dram_tensor("sinT", [128, T], BF16, kind="ExternalInput").ap()
    mask_d = nc.dram_tensor("masks", [128, 128], F8, kind="ExternalInput").ap()
    ones_d = nc.dram_tensor("ones", [128, 2, 128], F8, kind="ExternalInput").ap()
    out_d = nc.dram_tensor("out_partial", [T, HID], BF16, kind="ExternalOutput").ap()

    with tile.TileContext(nc) as tc:
        with tc.tile_pool(name="const", bufs=1) as cpool, \
             tc.tile_pool(name="acts", bufs=1) as apool, \
             tc.tile_pool(name="hstream", bufs=2) as hpool, \
             tc.tile_pool(name="rope", bufs=2) as tpool, \
             tc.tile_pool(name="exp", bufs=12) as epool, \
             tc.tile_pool(name="ctmp", bufs=2) as t2pool, \
             tc.tile_pool(name="outsb", bufs=3) as opool, \
             tc.tile_pool(name="qkvps", bufs=2, space="PSUM") as qkv_ps, \
             tc.tile_pool(name="sps", bufs=2, space="PSUM") as spool, \
             tc.tile_pool(name="cps", bufs=1, space="PSUM") as cpsp, \
             tc.tile_pool(name="dps", bufs=1, space="PSUM") as dpsp:

            w_sb = {k: cpool.tile([128, NPAIR, 2, 2, 128], F8,
                                  name=f"w_{k[0]}{k[1]}", tag=f"w_{k[0]}{k[1]}")
                    for k in w_d}
            wo_hi = cpool.tile([128, 2, HID], F8)
            wo_re = cpool.tile([128, 2, HID], F8)
            cos_sb = cpool.tile([128, T], BF16)
            sin_sb = cpool.tile([128, T], BF16)
            mask_sb = cpool.tile([128, 128], F8)
            ones_sb = cpool.tile([128, 2, 128], F8)

            qrT = [[apool.tile([128, TB], F32R, name=f"q{i}b{b}", tag=f"q{i}b{b}")
                    for b in range(NTB)] for i in range(QH)]
            krT = [apool.tile([128, TB], F32R, name=f"kb{b}", tag=f"kb{b}")
                   for b in range(NTB)]
            vbf = [apool.tile([128, 4, 128], BF16, name=f"vbf{b}", tag=f"vbf{b}")
                   for b in range(NTB)]
            v_hi = [apool.tile([128, 2, 2, 128], F8, name=f"vhb{b}", tag=f"vhb{b}")
                    for b in range(NTB)]
            v_re = [apool.tile([128, 2, 2, 128], F8, name=f"vrb{b}", tag=f"vrb{b}")
                    for b in range(NTB)]
            ctx_hi = [apool.tile([128, 2, TB], F8, name=f"chb{b}", tag=f"chb{b}")
                      for b in range(NTB)]
            ctx_re = [apool.tile([128, 2, TB], F8, name=f"crb{b}", tag=f"crb{b}")
                      for b in range(NTB)]

            h_hi_v = h_hi_d.rearrange("(j i p) t -> p j i t", i=2, p=128)
            h_re_v = h_re_d.rearrange("(j i p) t -> p j i t", i=2, p=128)

            h_tiles = {}
            qkv_state = {}
            att_state = {}
            rr = {"osb": 0}

            def u_dma_h(b, hv):
                """Load one h stream (hi or re) for block b as two half-slabs
                on the SP queue."""
                def run():
                    src = h_hi_v if hv == "hi" else h_re_v
                    for ab, j0 in (("A", 0), ("B", 4)):
                        t = hpool.tile([128, 4, 2, TB], F8, tag=f"h{hv}{ab}")
                        h_tiles[(b, hv, ab)] = t
                        nc.sync.dma_start(t[:], src[:, j0:j0 + 4, :, ts(b, TB)])
                return run

            def u_dma_w(hv, ab):
                def run():
                    nc.sync.dma_start(w_sb[(hv, ab)][:], w_d[(hv, ab)])
                return run

            def u_dma_tables(b):
                def run():
                    nc.sync.dma_start(cos_sb[:, ts(b, TB)], cos_d[:, ts(b, TB)])
                    nc.sync.dma_start(sin_sb[:, ts(b, TB)], sin_d[:, ts(b, TB)])
                    if b == 0:
                        nc.sync.dma_start(ones_sb[:], ones_d)
                        nc.sync.dma_start(mask_sb[:], mask_d)
                return run

            def u_dma_const1():
                def run():
                    nc.sync.dma_start(wo_hi[:], wo_hi_d)
                    nc.sync.dma_start(wo_re[:], wo_re_d)
                return run

            # ---------------- QKV stream ------------------------------------
            def u_qkv_mm(b, n, stream, jh):
                """One unit = 4 DoubleRow matmuls (j = jh*4 .. jh*4+3)."""
                def run():
                    ab = "A" if jh == 0 else "B"
                    wab = "A" if n < 2 else "B"
                    if stream == 0 and jh == 0:
                        qkv_state[(b, n)] = qkv_ps.tile(
                            [128, TB], F32, name=f"qkv{n}", tag="qkv")
                    ps = qkv_state[(b, n)]
                    wv, hv = [("hi", "hi"), ("hi", "re"), ("re", "hi")][stream]
                    wt = w_sb[(wv, wab)]
                    ht = h_tiles[(b, hv, ab)]
                    for jj in range(4):
                        j = jh * 4 + jj
                        nc.tensor.matmul(
                            ps[:], wt[:, j, :, n % 2, :], ht[:, jj],
                            perf_mode=DR,
                            start=(stream == 0 and j == 0),
                            stop=(stream == 2 and j == NPAIR - 1))
                return run

            def u_evict_rope(b, n):
                """Evict qkv psum n (q0/q1/k) and run its RoPE chain."""
                def run():
                    ps = qkv_state.pop((b, n))
                    x_sb = tpool.tile([128, TB], F32, tag=f"x{n}")
                    if n == 1:
                        nc.scalar.copy(x_sb[:], ps[:])
                    else:
                        nc.vector.tensor_copy(x_sb[:], ps[:])
                    dst = qrT[n][b] if n < QH else krT[b]
                    xsw = tpool.tile([128, TB], F32, tag=f"xsw{n}")
                    nc.sync.dma_start(xsw[0:64, :], x_sb[64:128, :])
                    nc.sync.dma_start(xsw[64:128, :], x_sb[0:64, :])
                    t2 = tpool.tile([128, TB], F32, tag=f"t2{n}")
                    m1 = tpool.tile([128, TB], F32, tag=f"m1{n}")
                    nc.gpsimd.tensor_tensor(out=t2[:], in0=xsw[:],
                                            in1=sin_sb[:, ts(b, TB)], op=OP.mult)
                    nc.vector.tensor_tensor(out=m1[:], in0=x_sb[:],
                                            in1=cos_sb[:, ts(b, TB)], op=OP.mult)
                    nc.vector.tensor_tensor(out=dst[:], in0=m1[:], in1=t2[:],
                                            op=OP.add)
                return run

            def u_evict_v(b):
                def run():
                    ps = qkv_state.pop((b, 3))
                    x_sb = tpool.tile([128, TB], BF16, tag="xv")
                    nc.vector.tensor_scalar_mul(x_sb[:], ps[:], S_ADJ)
                    qkv_state[("vT", b)] = x_sb
                return run

            def u_vtrans(b, jj):
                def run():
                    vT_sb = qkv_state[("vT", b)]
                    nc.sync.dma_start_transpose(vbf[b][:, jj, :],
                                                  vT_sb[:, ts(jj, 128)])
                return run

            def u_vquant(b, jj):
                def run():
                    p, s = jj // 2, jj % 2
                    nc.gpsimd.tensor_copy(v_hi[b][:, p, s, :], vbf[b][:, jj, :])
                    nc.gpsimd.tensor_tensor(out=v_re[b][:, p, s, :],
                                            in0=vbf[b][:, jj, :],
                                            in1=v_hi[b][:, p, s, :],
                                            op=OP.subtract)
                return run

            # ---------------- attention stream ------------------------------
            def qlo_of(bb, p):
                """Exact-causal moving-range start for pair p of block bb
                (clamped so fp32r keeps >=256 moving columns)."""
                k0 = 2 * p - 4 * bb          # first local key tile of the pair
                if k0 < 0:
                    return 0
                return min(128 * k0, TB - 256)

            def u_score_pair(bb, qh, p):
                """Two fp32r score matmuls + one paired exp (+ diag masks)."""
                def run():
                    st = att_state.setdefault((bb, qh), {})
                    s_pair = spool.tile([128, 2, TB], F32, name="s_pair",
                                        tag="sps")
                    e_pair = epool.tile([128, 2, TB], F8, name="e_pair",
                                        tag="exp")
                    st[("e", p)] = e_pair
                    q0 = qlo_of(bb, p)
                    for s in range(2):
                        kt = 2 * p + s
                        nc.tensor.matmul(s_pair[:, s, q0:TB],
                                         krT[kt // 4][:, ts(kt % 4, 128)],
                                         qrT[qh][bb][:, q0:TB],
                                         start=True, stop=True)
                    nc.scalar.activation(e_pair[:, :, q0:TB],
                                         s_pair[:, :, q0:TB], AF.Exp,
                                         scale=SCALE)
                    if 2 * p + 1 >= 4 * bb:   # diagonal pair: causal masks
                        for s in range(2):
                            kt_l = 2 * p + s - 4 * bb
                            c0 = 128 * kt_l
                            nc.gpsimd.tensor_tensor(
                                out=e_pair[:, s, c0:c0 + 128],
                                in0=e_pair[:, s, c0:c0 + 128],
                                in1=mask_sb[:], op=OP.mult)
                            if s == 1 and c0 > q0:
                                nc.gpsimd.memset(e_pair[:, 1, q0:c0], 0.0)
                return run

            def u_consume(bb, qh, p, npair):
                def run():
                    st = att_state[(bb, qh)]
                    if p == 0:
                        st["ctx"] = cpsp.tile([128, TB], F32, name="ctx_ps")
                        st["den"] = dpsp.tile([128, TB], F32, name="den_ps")
                    e_pair = st.pop(("e", p))
                    first = (p == 0)
                    last = (p == npair - 1)
                    q0 = qlo_of(bb, p)
                    vb, vp = p // 2, p % 2
                    nc.tensor.matmul(st["ctx"][:, q0:TB], v_hi[vb][:, vp, :, :],
                                     e_pair[:, :, q0:TB], perf_mode=DR,
                                     start=first, stop=False)
                    nc.tensor.matmul(st["ctx"][:, q0:TB], v_re[vb][:, vp, :, :],
                                     e_pair[:, :, q0:TB], perf_mode=DR,
                                     start=False, stop=last)
                    nc.tensor.matmul(st["den"][:, q0:TB], ones_sb[:],
                                     e_pair[:, :, q0:TB], perf_mode=DR,
                                     start=first, stop=last)
                return run

            def u_ctx1(bb, qh):
                def run():
                    st = att_state[(bb, qh)]
                    recip = t2pool.tile([128, TB], F32, tag="recip")
                    c32 = t2pool.tile([128, TB], F32, tag="c32")
                    nc.vector.reciprocal(recip[:], st["den"][:])
                    nc.vector.tensor_tensor(out=c32[:], in0=st["ctx"][:],
                                            in1=recip[:], op=OP.mult)
                    st["c32"] = c32
                return run

            def u_ctx2(bb, qh):
                def run():
                    st = att_state.pop((bb, qh))
                    c32 = st["c32"]
                    nc.vector.tensor_copy(ctx_hi[bb][:, qh, :], c32[:])
                    nc.gpsimd.tensor_tensor(out=ctx_re[bb][:, qh, :],
                                            in0=c32[:],
                                            in1=ctx_hi[bb][:, qh, :],
                                            op=OP.subtract)
                return run

            def att_units(bb, qh, part):
                """nd: scores+consumes for non-diagonal pairs; nd_s/nd_c:
                scores-only / consumes-only variants (e pairs buffered in
                epool between them); d: diagonal pairs + softmax chain."""
                npair = 2 * (bb + 1)
                if part in ("nd", "nd_s", "nd_c"):
                    pairs = range(0, 2 * bb)
                else:
                    pairs = range(2 * bb, npair)
                units = []
                for p in pairs:
                    if part != "nd_c":
                        units.append(u_score_pair(bb, qh, p))
                    if part != "nd_s":
                        units.append(u_consume(bb, qh, p, npair))
                if part == "d":
                    units.append(u_ctx1(bb, qh))
                    units.append(u_ctx2(bb, qh))
                return units

            # ---------------- output-projection stream ----------------------
            def out_units(bb):
                units = []
                st = {}

                def u_alloc(tt, st=st):
                    def run():
                        st[tt] = opool.tile([128, 2, 2, TB], BF16, name="o_sb")
                    return run

                def u_proj(tt, half, bb=bb, st=st):
                    def run():
                        ps = spool.tile([128, 2, TB], F32, name="ps_o",
                                        tag="sps")
                        ch = ctx_hi[bb][:, :, ts(tt % 4, 128)]
                        cr = ctx_re[bb][:, :, ts(tt % 4, 128)]
                        for s in range(2):
                            n = 2 * half + s
                            nc.tensor.matmul(ps[:, s, :], ch,
                                             wo_hi[:, :, ts(n, 512)],
                                             perf_mode=DR, start=True, stop=False)
                            nc.tensor.matmul(ps[:, s, :], ch,
                                             wo_re[:, :, ts(n, 512)],
                                             perf_mode=DR, start=False, stop=False)
                            nc.tensor.matmul(ps[:, s, :], cr,
                                             wo_hi[:, :, ts(n, 512)],
                                             perf_mode=DR, start=False, stop=True)
                        rr["osb"] += 1
                        if rr["osb"] % 2:
                            nc.vector.tensor_copy(st[tt][:, half, :, :], ps[:])
                        else:
                            nc.scalar.copy(st[tt][:, half, :, :], ps[:])
                    return run

                def u_odma(tt, st=st):
                    def run():
                        nc.sync.dma_start(out_d[ts(tt, 128), :], st.pop(tt)[:])
                    return run

                # emit the DMA for tile tt after the next tile's first proj
                # so the SP queue never head-of-line blocks on the evict.
                pend = []
                for tt in range(4 * bb, 4 * bb + 4):
                    units.append(u_alloc(tt))
                    units.append(u_proj(tt, 0))
                    if pend:
                        units.append(pend.pop())
                    units.append(u_proj(tt, 1))
                    pend.append(u_odma(tt))
                units += pend
                return units

            # ---------------- merged emission --------------------------------
            def qkv_pass(b, ns):
                units = []
                for stream in range(3):
                    for n in ns:
                        for jh in range(2):
                            units.append(u_qkv_mm(b, n, stream, jh))
                return units

            for b in range(NTB):
                # ---- pass A (q heads) ----
                mainA = []
                if b == 0:
                    mainA.append(u_dma_w("hi", "A"))
                    mainA.append(u_dma_h(0, "hi"))
                    mainA.append(u_dma_h(0, "re"))
                    mainA.append(u_dma_w("re", "A"))
                mainA += qkv_pass(b, (0, 1))
                if b == 0:
                    mainA.insert(7, u_dma_w("hi", "B"))
                    mainA.insert(8, u_dma_w("re", "B"))
                else:
                    mainA.insert(6, u_dma_h(b + 1, "hi") if b + 1 < NTB
                                 else (lambda: None))
                mainA.append(u_dma_tables(b))
                if b + 1 < NTB:
                    if b == 0:
                        mainA.append(u_dma_h(b + 1, "hi"))
                    mainA.append(u_dma_h(b + 1, "re"))
                fillerA = []
                if b > 0:
                    fillerA += [u_vtrans(b - 1, jj) for jj in range(4)]
                    fillerA += [u_vquant(b - 1, jj) for jj in range(4)]
                    fillerA += att_units(b - 1, 0, "d")
                    fillerA += att_units(b - 1, 1, "nd_c")
                    fillerA += att_units(b - 1, 1, "d")
                _interleave(mainA, fillerA)
                # ---- RoPE for q heads ----
                u_evict_rope(b, 0)()
                u_evict_rope(b, 1)()
                # ---- pass B (k, v) ----
                mainB = qkv_pass(b, (2, 3))
                if b == 0:
                    mainB.append(u_dma_const1())
                fillerB = []
                if b > 0:
                    fillerB += out_units(b - 1)
                fillerB += att_units(b, 0, "nd")
                fillerB += att_units(b, 1, "nd_s")
                _interleave(mainB, fillerB)
                u_evict_rope(b, 2)()
                u_evict_v(b)()

            # ---- tail: attention(last) + out(last) ----
            bl = NTB - 1
            tail = [u_vtrans(bl, jj) for jj in range(4)]
            tail += [u_vquant(bl, jj) for jj in range(4)]
            tail += att_units(bl, 0, "d")
            tail += att_units(bl, 1, "nd_c")
            tail += att_units(bl, 1, "d")
            tail += out_units(bl)
            for u in tail:
                u()

    nc.compile()
    return nc


_NC_CACHE = None


def _get_nc():
    global _NC_CACHE
    if _NC_CACHE is None:
        _NC_CACHE = _build()
    return _NC_CACHE


def _f8(x):
    return np.asarray(x, np.float32).astype(NPF8)


def _host_tables(position_ids: np.ndarray):
    pos = np.asarray(position_ids, np.float32)
    inv_freq = (1.0 / (THETA ** (np.arange(0, D, 2, dtype=np.float32) / D)))
    ang = pos[:, None] * inv_freq[None, :]          # [T, 64] f32
    dq = np.float32(1.0 / (S_H * S_W))              # QKV dequant folded in
    cos = (np.cos(ang).T * dq).astype(np.float32)   # [64, T]
    sin = (np.sin(ang).T * dq).astype(np.float32)
    cosT = np.concatenate([cos, cos], axis=0).astype(NPBF)   # [128, T]
    sinT = np.concatenate([-sin, sin], axis=0).astype(NPBF)
    return cosT, sinT


def _host_mask():
    r = np.arange(128)[:, None]
    c = np.arange(128)[None, :]
    return _f8((c - r >= 0).astype(np.float32))     # [128, 128] triangle


def _pack_w(w_local_scaled: np.ndarray):
    """[2048, 512] (already scaled) -> hi/res packed [128, 8, 2, 4, 128]."""
    hi = _f8(w_local_scaled)
    re = _f8(w_local_scaled - hi.astype(np.float32))

    def pack(a):
        # row r = j*256 + i*128 + p ; col = n*128 + c
        return np.ascontiguousarray(
            a.reshape(NPAIR, 2, 128, 4, 128).transpose(2, 0, 1, 3, 4))
    return pack(hi), pack(re)


def kernel(hidden_states, position_ids, Wqkv, Wo):
    hidden_states = np.asarray(hidden_states, np.float32)
    Wqkv = np.asarray(Wqkv, np.float32)
    Wo = np.asarray(Wo, np.float32)

    nc = _get_nc()

    hT_s = hidden_states.T * np.float32(S_H)
    h_hi = _f8(hT_s)
    h_re = _f8(hT_s - h_hi.astype(np.float32))
    cosT, sinT = _host_tables(position_ids)
    mask = _host_mask()
    ones = np.full((128, 2, 128), ALPHA, np.float32).astype(NPF8)

    wq = Wqkv[:, : H * D]
    wk = Wqkv[:, H * D: (H + KV) * D]
    wv = Wqkv[:, (H + KV) * D:]

    in_maps = []
    for c in range(N_CORES):
        kvh = (c * QH) // (H // KV)
        w_local = np.concatenate(
            [wq[:, (c * QH) * D: (c * QH + 1) * D],
             wq[:, (c * QH + 1) * D: (c * QH + 2) * D],
             wk[:, kvh * D: (kvh + 1) * D],
             wv[:, kvh * D: (kvh + 1) * D]], axis=1) * np.float32(S_W)
        w_hi, w_re = _pack_w(w_local)   # [128, 8, 2, 4, 128]
        w_split = {
            "w_hiA": np.ascontiguousarray(w_hi[:, :, :, 0:2, :]),
            "w_hiB": np.ascontiguousarray(w_hi[:, :, :, 2:4, :]),
            "w_reA": np.ascontiguousarray(w_re[:, :, :, 0:2, :]),
            "w_reB": np.ascontiguousarray(w_re[:, :, :, 2:4, :]),
        }
        wo_local = Wo[c * QH * D: (c + 1) * QH * D, :] * np.float32(S_WO)
        wo_hi = _f8(wo_local)
        wo_re = _f8(wo_local - wo_hi.astype(np.float32))
        # [2*128, HID] -> [128, 2, HID] (slot = head)
        wo_hi = np.ascontiguousarray(wo_hi.reshape(2, 128, HID).transpose(1, 0, 2))
        wo_re = np.ascontiguousarray(wo_re.reshape(2, 128, HID).transpose(1, 0, 2))
        in_maps.append({
            "h_hi": h_hi, "h_re": h_re,
            **w_split,
            "wo_hi": wo_hi, "wo_re": wo_re,
            "cosT": cosT, "sinT": sinT, "masks": mask,
            "ones": ones,
        })

    res = bass_utils.run_bass_kernel_spmd(nc, in_maps,
                                          core_ids=list(range(N_CORES)))
    parts = np.stack([res.results[c]["out_partial"].astype(np.float32)
                      for c in range(N_CORES)], 0)
    return parts.sum(axis=0, dtype=np.float32) * np.float32(1.0 / S_OUT)


# revision 57
# speedup vs baseline: 1.3865x; 1.0311x over previous
"""Bass/Trainium2 kernel for BailingAttention (GQA prefill, causal, RoPE).

Sharding: tensor-parallel over heads across 8 NeuronCores. Each core computes
2 query heads + its group's shared KV head end-to-end and writes a partial
[T, HID] output (bf16); the host sums the 8 partials (row-parallel
all-reduce) and applies the global dequant scale.

Numerics (rel tolerance 2e-2; this lands ~1e-2):
  - QKV and output projections are 3-term split-fp8 DoubleRow matmuls:
    x ~ hi + res, both e4m3 in SHARED scale units, so hi@hi + hi@res +
    res@hi accumulates in one PSUM group at ~0.1% error and 3/4 the fp32r
    PE cost (DoubleRow = 0.5 cyc/row over a 2x128 contraction).
  - Scores stay fp32r (the exp is error-sensitive).
  - exp outputs (e) are e4m3; PV and the softmax denominator run DoubleRow
    over adjacent key-tile PAIRS (e pair tiles [128,2,512]); v is hi+res.
  - All scales are powers of two, folded into host tables (cos/sin carry
    the QKV dequant, the denominator 'ones' stationary carries the ctx
    scale, the final dequant happens host-side during the partial sum).

Schedule: one software-pipelined emission stream. Per 512-token block b:
pass A (q heads) QKV matmuls interleaved with attention(b-1) leftovers and
out-proj(b-1); RoPE(q); pass B (k/v) interleaved with attention(b) head 0's
non-diagonal work. One attention head is in flight at a time so PSUM fits:
QKV ring 2 banks + score-pair ring 4 + ctx 1 + den 1 = 8.

Engine discipline (each engine's FIFO order gates its consumers):
  ACT  = exps only, plus issue-only DMAs (xsw swaps, v DMA-transpose).
  DVE  = psum evicts (q0/k/v), RoPE m1+add, softmax recip/c32/ctx_hi,
         half the out-proj evicts.
  Pool = q1 evict, RoPE t2, causal tri-masks+memsets (shrunk to [128,128]),
         ctx_re, v quantize.
  SP   = all dependency-free input DMAs + output DMAs (emitted late).
Diagonal score pairs are exact-causal: the moving range shrinks to >=256
columns, exp shrinks with it, masks act on [128,128] triangles only.
"""

import numpy as np
import ml_dtypes

import concourse.bass as bass
import concourse.mybir as mybir
import concourse.tile as tile
from concourse import bacc, bass_utils
from concourse.bass import ts

F32 = mybir.dt.float32
F32R = mybir.dt.float32r
F8 = mybir.dt.float8e4
BF16 = mybir.dt.bfloat16
AF = mybir.ActivationFunctionType
OP = mybir.AluOpType
DR = mybir.MatmulPerfMode.DoubleRow
NPF8 = ml_dtypes.float8_e4m3
NPBF = ml_dtypes.bfloat16

H, KV, D, HID, T = 16, 4, 128, 2048, 2048
THETA = 10000.0
N_CORES = 8
QH = H // N_CORES            # query heads per core = 2
TB = 512                     # token block (matmul moving N)
NTB = T // TB                # 4
NPAIR = HID // 256           # 8 contraction pairs for QKV
SCALE = float(D) ** -0.5

S_H = 32.0                   # hidden quant scale
S_W = 1024.0                 # Wqkv quant scale
S_V = 4.0                    # on-device v scale (v tiles = 4*v_true)
S_ADJ = S_V / (S_H * S_W)    # v psum -> scaled-v evict factor
ALPHA = 0.125                # ones value: ctx_hi = (S_V/ALPHA)*ctx = 32*ctx
S_WO = 1024.0                # Wo quant scale
S_OUT = (S_V / ALPHA) * S_WO  # host-side dequant of the partial outputs


def _riffle(a, b):
    """Proportionally merge two unit lists into one."""
    if not a or not b:
        return a + b
    out = []
    r = len(a) / len(b)
    bi = 0
    acc = 0.0
    for u in a:
        out.append(u)
        acc += 1.0
        while bi < len(b) and acc >= r:
            out.append(b[bi])
            bi += 1
            acc -= r
    out.extend(b[bi:])
    return out


def _interleave(main, filler):
    """Emit `main` and `filler` unit lists proportionally merged."""
    if not filler or not main:
        for u in main + filler:
            u()
        return
    r = len(main) / len(filler)
    fi = 0
    acc = 0.0
    for u in main:
        u()
        acc += 1.0
        while fi < len(filler) and acc >= r:
            filler[fi]()
            fi += 1
            acc -= r
    while fi < len(filler):
        filler[fi]()
        fi += 1


def _build():
    nc = bacc.Bacc("TRN2", target_bir_lowering=False, debug=False,
                   num_devices=N_CORES)

    h_hi_d = nc.dram_tensor("h_hi", [HID, T], F8, kind="ExternalInput").ap()
    h_re_d = nc.dram_tensor("h_re", [HID, T], F8, kind="ExternalInput").ap()
    # w split by output-column pair: A = n in (0,1) [q heads], B = n in (2,3)
    w_d = {}
    for hv in ("hi", "re"):
        for ab in ("A", "B"):
            w_d[(hv, ab)] = nc.dram_tensor(
                f"w_{hv}{ab}", [128, NPAIR, 2, 2, 128], F8,
                kind="ExternalInput").ap()
    wo_hi_d = nc.dram_tensor("wo_hi", [128, 2, HID], F8,
                             kind="ExternalInput").ap()
    wo_re_d = nc.dram_tensor("wo_re", [128, 2, HID], F8,
                             kind="ExternalInput").ap()
    cos_d = nc.dram_tensor("cosT", [128, T], BF16, kind="ExternalInput").ap()
    sin_d = nc.dram_tensor("sinT", [128, T], BF16, kind="ExternalInput").ap()
    mask_d = nc.dram_tensor("masks", [128, 128], F8, kind="ExternalInput").ap()
    ones_d = nc.dram_tensor("ones", [128, 2, 128], F8, kind="ExternalInput").ap()
    out_d = nc.dram_tensor("out_partial", [T, HID], BF16, kind="ExternalOutput").ap()

    with tile.TileContext(nc) as tc:
        with tc.tile_pool(name="const", bufs=1) as cpool, \
             tc.tile_pool(name="acts", bufs=1) as apool, \
             tc.tile_pool(name="hstream", bufs=2) as hpool, \
             tc.tile_pool(name="rope", bufs=2) as tpool, \
             tc.tile_pool(name="exp", bufs=12) as epool, \
             tc.tile_pool(name="ctmp", bufs=3) as t2pool, \
             tc.tile_pool(name="outsb", bufs=3) as opool, \
             tc.tile_pool(name="qkvps", bufs=2, space="PSUM") as qkv_ps, \
             tc.tile_pool(name="sps", bufs=2, space="PSUM") as spool, \
             tc.tile_pool(name="cps", bufs=1, space="PSUM") as cpsp, \
             tc.tile_pool(name="dps", bufs=1, space="PSUM") as dpsp:

            w_sb = {k: cpool.tile([128, NPAIR, 2, 2, 128], F8,
                                  name=f"w_{k[0]}{k[1]}", tag=f"w_{k[0]}{k[1]}")
                    for k in w_d}
            wo_hi = cpool.tile([128, 2, HID], F8)
            wo_re = cpool.tile([128, 2, HID], F8)
            cos_sb = cpool.tile([128, T], BF16)
            sin_sb = cpool.tile([128, T], BF16)
            mask_sb = cpool.tile([128, 128], F8)
            ones_sb = cpool.tile([128, 2, 128], F8)

            qrT = [[apool.tile([128, TB], F32R, name=f"q{i}b{b}", tag=f"q{i}b{b}")
                    for b in range(NTB)] for i in range(QH)]
            krT = [apool.tile([128, TB], F32R, name=f"kb{b}", tag=f"kb{b}")
                   for b in range(NTB)]
            vbf = [apool.tile([128, 4, 128], BF16, name=f"vbf{b}", tag=f"vbf{b}")
                   for b in range(NTB)]
            v_hi = [apool.tile([128, 2, 2, 128], F8, name=f"vhb{b}", tag=f"vhb{b}")
                    for b in range(NTB)]
            v_re = [apool.tile([128, 2, 2, 128], F8, name=f"vrb{b}", tag=f"vrb{b}")
                    for b in range(NTB)]
            ctx_hi = [apool.tile([128, 2, TB], F8, name=f"chb{b}", tag=f"chb{b}")
                      for b in range(NTB)]
            ctx_re = [apool.tile([128, 2, TB], F8, name=f"crb{b}", tag=f"crb{b}")
                      for b in range(NTB)]

            h_hi_v = h_hi_d.rearrange("(j i p) t -> p j i t", i=2, p=128)
            h_re_v = h_re_d.rearrange("(j i p) t -> p j i t", i=2, p=128)

            h_tiles = {}
            qkv_state = {}
            att_state = {}
            rr = {"osb": 0}

            def u_dma_h(b, hv):
                """Load one h stream (hi or re) for block b as two half-slabs
                on the SP queue."""
                def run():
                    src = h_hi_v if hv == "hi" else h_re_v
                    for ab, j0 in (("A", 0), ("B", 4)):
                        t = hpool.tile([128, 4, 2, TB], F8, tag=f"h{hv}{ab}")
                        h_tiles[(b, hv, ab)] = t
                        nc.sync.dma_start(t[:], src[:, j0:j0 + 4, :, ts(b, TB)])
                return run

            def u_dma_w(hv, ab):
                def run():
                    nc.sync.dma_start(w_sb[(hv, ab)][:], w_d[(hv, ab)])
                return run

            def u_dma_tables(b):
                def run():
                    nc.sync.dma_start(cos_sb[:, ts(b, TB)], cos_d[:, ts(b, TB)])
                    nc.sync.dma_start(sin_sb[:, ts(b, TB)], sin_d[:, ts(b, TB)])
                    if b == 0:
                        nc.sync.dma_start(ones_sb[:], ones_d)
                        nc.sync.dma_start(mask_sb[:], mask_d)
                return run

            def u_dma_const1():
                def run():
                    nc.sync.dma_start(wo_hi[:], wo_hi_d)
                    nc.sync.dma_start(wo_re[:], wo_re_d)
                return run

            # ---------------- QKV stream ------------------------------------
            def u_qkv_mm(b, n, stream, jh):
                """One unit = 4 DoubleRow matmuls (j = jh*4 .. jh*4+3)."""
                def run():
                    ab = "A" if jh == 0 else "B"
                    wab = "A" if n < 2 else "B"
                    if stream == 0 and jh == 0:
                        qkv_state[(b, n)] = qkv_ps.tile(
                            [128, TB], F32, name=f"qkv{n}", tag="qkv")
                    ps = qkv_state[(b, n)]
                    wv, hv = [("hi", "hi"), ("hi", "re"), ("re", "hi")][stream]
                    wt = w_sb[(wv, wab)]
                    ht = h_tiles[(b, hv, ab)]
                    for jj in range(4):
                        j = jh * 4 + jj
                        nc.tensor.matmul(
                            ps[:], wt[:, j, :, n % 2, :], ht[:, jj],
                            perf_mode=DR,
                            start=(stream == 0 and j == 0),
                            stop=(stream == 2 and j == NPAIR - 1))
                return run

            def u_evict_rope(b, n):
                """Evict qkv psum n (q0/q1/k) and run its RoPE chain."""
                def run():
                    ps = qkv_state.pop((b, n))
                    x_sb = tpool.tile([128, TB], F32, tag=f"x{n}")
                    if n == 1:
                        nc.scalar.copy(x_sb[:], ps[:])
                    else:
                        nc.vector.tensor_copy(x_sb[:], ps[:])
                    dst = qrT[n][b] if n < QH else krT[b]
                    xsw = tpool.tile([128, TB], F32, tag=f"xsw{n}")
                    nc.sync.dma_start(xsw[0:64, :], x_sb[64:128, :])
                    nc.sync.dma_start(xsw[64:128, :], x_sb[0:64, :])
                    t2 = tpool.tile([128, TB], F32, tag=f"t2{n}")
                    m1 = tpool.tile([128, TB], F32, tag=f"m1{n}")
                    nc.vector.tensor_tensor(out=m1[:], in0=x_sb[:],
                                            in1=cos_sb[:, ts(b, TB)], op=OP.mult)
                    nc.gpsimd.tensor_tensor(out=t2[:], in0=xsw[:],
                                            in1=sin_sb[:, ts(b, TB)], op=OP.mult)
                    nc.vector.tensor_tensor(out=dst[:], in0=m1[:], in1=t2[:],
                                            op=OP.add)
                return run

            def u_evict_v(b):
                def run():
                    ps = qkv_state.pop((b, 3))
                    x_sb = tpool.tile([128, TB], BF16, tag="xv")
                    nc.vector.tensor_scalar_mul(x_sb[:], ps[:], S_ADJ)
                    qkv_state[("vT", b)] = x_sb
                return run

            def u_vtrans(b, jj):
                def run():
                    vT_sb = qkv_state[("vT", b)]
                    nc.sync.dma_start_transpose(vbf[b][:, jj, :],
                                                  vT_sb[:, ts(jj, 128)])
                return run

            def u_vquant(b, jj):
                def run():
                    p, s = jj // 2, jj % 2
                    nc.vector.tensor_copy(v_hi[b][:, p, s, :], vbf[b][:, jj, :])
                    nc.vector.tensor_tensor(out=v_re[b][:, p, s, :],
                                            in0=vbf[b][:, jj, :],
                                            in1=v_hi[b][:, p, s, :],
                                            op=OP.subtract)
                return run

            # ---------------- attention stream ------------------------------
            def qlo_of(bb, p):
                """Exact-causal moving-range start for pair p of block bb
                (clamped so fp32r keeps >=256 moving columns)."""
                k0 = 2 * p - 4 * bb          # first local key tile of the pair
                if k0 < 0:
                    return 0
                return min(128 * k0, TB - 256)

            def u_score_pair(bb, qh, p):
                """Two fp32r score matmuls + one paired exp (+ diag masks)."""
                def run():
                    st = att_state.setdefault((bb, qh), {})
                    s_pair = spool.tile([128, 2, TB], F32, name="s_pair",
                                        tag="sps")
                    e_pair = epool.tile([128, 2, TB], F8, name="e_pair",
                                        tag="exp")
                    st[("e", p)] = e_pair
                    q0 = qlo_of(bb, p)
                    for s in range(2):
                        kt = 2 * p + s
                        nc.tensor.matmul(s_pair[:, s, q0:TB],
                                         krT[kt // 4][:, ts(kt % 4, 128)],
                                         qrT[qh][bb][:, q0:TB],
                                         start=True, stop=True)
                    nc.scalar.activation(e_pair[:, :, q0:TB],
                                         s_pair[:, :, q0:TB], AF.Exp,
                                         scale=SCALE)
                    if 2 * p + 1 >= 4 * bb:   # diagonal pair: causal masks
                        for s in range(2):
                            kt_l = 2 * p + s - 4 * bb
                            c0 = 128 * kt_l
                            nc.gpsimd.tensor_tensor(
                                out=e_pair[:, s, c0:c0 + 128],
                                in0=e_pair[:, s, c0:c0 + 128],
                                in1=mask_sb[:], op=OP.mult)
                            if s == 1 and c0 > q0:
                                nc.gpsimd.memset(e_pair[:, 1, q0:c0], 0.0)
                return run

            def u_consume(bb, qh, p, npair):
                def run():
                    st = att_state[(bb, qh)]
                    if p == 0:
                        st["ctx"] = cpsp.tile([128, TB], F32, name="ctx_ps")
                        st["den"] = dpsp.tile([128, TB], F32, name="den_ps")
                    e_pair = st.pop(("e", p))
                    first = (p == 0)
                    last = (p == npair - 1)
                    q0 = qlo_of(bb, p)
                    vb, vp = p // 2, p % 2
                    nc.tensor.matmul(st["ctx"][:, q0:TB], v_hi[vb][:, vp, :, :],
                                     e_pair[:, :, q0:TB], perf_mode=DR,
                                     start=first, stop=False)
                    nc.tensor.matmul(st["ctx"][:, q0:TB], v_re[vb][:, vp, :, :],
                                     e_pair[:, :, q0:TB], perf_mode=DR,
                                     start=False, stop=last)
                    nc.tensor.matmul(st["den"][:, q0:TB], ones_sb[:],
                                     e_pair[:, :, q0:TB], perf_mode=DR,
                                     start=first, stop=last)
                return run

            def u_ctx1(bb, qh):
                def run():
                    st = att_state[(bb, qh)]
                    recip = t2pool.tile([128, TB], F32, tag="recip")
                    c32 = t2pool.tile([128, TB], F32, tag="c32")
                    nc.vector.reciprocal(recip[:], st["den"][:])
                    nc.vector.tensor_tensor(out=c32[:], in0=st["ctx"][:],
                                            in1=recip[:], op=OP.mult)
                    st["c32"] = c32
                return run

            def u_ctx2(bb, qh):
                def run():
                    st = att_state.pop((bb, qh))
                    c32 = st["c32"]
                    nc.vector.tensor_copy(ctx_hi[bb][:, qh, :], c32[:])
                    nc.gpsimd.tensor_tensor(out=ctx_re[bb][:, qh, :],
                                            in0=c32[:],
                                            in1=ctx_hi[bb][:, qh, :],
                                            op=OP.subtract)
                return run

            def att_units(bb, qh, part):
                """nd: scores+consumes for non-diagonal pairs; nd_s/nd_c:
                scores-only / consumes-only variants (e pairs buffered in
                epool between them); d: diagonal pairs + softmax chain."""
                npair = 2 * (bb + 1)
                if part in ("nd", "nd_s", "nd_c"):
                    pairs = range(0, 2 * bb)
                else:
                    pairs = range(2 * bb, npair)
                units = []
                prev = None
                for p in pairs:
                    if part != "nd_c":
                        units.append(u_score_pair(bb, qh, p))
                    if part != "nd_s":
                        if part in ("nd", "d") and prev is not None:
                            units.append(prev)
                        if part in ("nd", "d"):
                            prev = u_consume(bb, qh, p, npair)
                        else:
                            units.append(u_consume(bb, qh, p, npair))
                if prev is not None:
                    units.append(prev)
                if part == "d":
                    units.append(u_ctx1(bb, qh))
                    units.append(u_ctx2(bb, qh))
                return units

            # ---------------- output-projection stream ----------------------
            def out_units(bb):
                units = []
                st = {}

                def u_alloc(tt, st=st):
                    def run():
                        st[tt] = opool.tile([128, 2, 2, TB], BF16, name="o_sb")
                    return run

                def u_proj(tt, half, bb=bb, st=st):
                    def run():
                        ps = spool.tile([128, 2, TB], F32, name="ps_o",
                                        tag="sps")
                        ch = ctx_hi[bb][:, :, ts(tt % 4, 128)]
                        cr = ctx_re[bb][:, :, ts(tt % 4, 128)]
                        for s in range(2):
                            n = 2 * half + s
                            nc.tensor.matmul(ps[:, s, :], ch,
                                             wo_hi[:, :, ts(n, 512)],
                                             perf_mode=DR, start=True, stop=False)
                            nc.tensor.matmul(ps[:, s, :], ch,
                                             wo_re[:, :, ts(n, 512)],
                                             perf_mode=DR, start=False, stop=False)
                            nc.tensor.matmul(ps[:, s, :], cr,
                                             wo_hi[:, :, ts(n, 512)],
                                             perf_mode=DR, start=False, stop=True)
                        rr["osb"] += 1
                        if rr["osb"] % 2:
                            nc.vector.tensor_copy(st[tt][:, half, :, :], ps[:])
                        else:
                            nc.scalar.copy(st[tt][:, half, :, :], ps[:])
                    return run

                def u_odma(tt, half, st=st):
                    def run():
                        o = st[tt] if half == 0 else st.pop(tt)
                        nc.sync.dma_start(
                            out_d[ts(tt, 128), ts(half, 1024)], o[:, half])
                    return run

                # emit the DMA for tile tt after the next tile's first proj
                # so the SP queue never head-of-line blocks on the evict.
                pend = []
                for tt in range(4 * bb, 4 * bb + 4):
                    units.append(u_alloc(tt))
                    units.append(u_proj(tt, 0))
                    if pend:
                        units.append(pend.pop(0))
                    units.append(u_proj(tt, 1))
                    pend.append(u_odma(tt, 0))
                    pend.append(u_odma(tt, 1))
                units += pend
                return units

            # ---------------- merged emission --------------------------------
            def qkv_pass(b, ns):
                units = []
                for stream in range(3):
                    for n in ns:
                        for jh in range(2):
                            units.append(u_qkv_mm(b, n, stream, jh))
                return units

            for b in range(NTB):
                # ---- pass A (q heads) ----
                mainA = []
                if b == 0:
                    mainA.append(u_dma_w("hi", "A"))
                    mainA.append(u_dma_h(0, "hi"))
                    mainA.append(u_dma_h(0, "re"))
                    mainA.append(u_dma_w("re", "A"))
                mainA += qkv_pass(b, (0, 1))
                if b == 0:
                    mainA.insert(7, u_dma_w("hi", "B"))
                    mainA.insert(8, u_dma_w("re", "B"))
                else:
                    mainA.insert(6, u_dma_h(b + 1, "hi") if b + 1 < NTB
                                 else (lambda: None))
                mainA.append(u_dma_tables(b))
                if b + 1 < NTB:
                    if b == 0:
                        mainA.append(u_dma_h(b + 1, "hi"))
                    mainA.append(u_dma_h(b + 1, "re"))
                fillerA = []
                if b > 0:
                    fillerA += [u_vtrans(b - 1, jj) for jj in range(4)]
                    fillerA += [u_vquant(b - 1, jj) for jj in range(4)]
                    fillerA += att_units(b - 1, 0, "d")
                    fillerA += att_units(b - 1, 1, "nd_c")
                    fillerA += att_units(b - 1, 1, "d")
                _interleave(mainA, fillerA)
                # ---- RoPE for q heads ----
                u_evict_rope(b, 0)()
                u_evict_rope(b, 1)()
                # ---- pass B (k, v) ----
                mainB = qkv_pass(b, (2, 3))
                if b == 0:
                    mainB.append(u_dma_const1())
                att_b = att_units(b, 0, "nd") + att_units(b, 1, "nd_s")
                fillerB = _riffle(out_units(b - 1) if b > 0 else [], att_b)
                _interleave(mainB, fillerB)
                u_evict_rope(b, 2)()
                u_evict_v(b)()

            # ---- tail: attention(last) + out(last) ----
            bl = NTB - 1
            tail = [u_vtrans(bl, jj) for jj in range(4)]
            tail += [u_vquant(bl, jj) for jj in range(4)]
            tail += att_units(bl, 0, "d")
            tail += att_units(bl, 1, "nd_c")
            tail += att_units(bl, 1, "d")
            tail += out_units(bl)
            for u in tail:
                u()

    nc.compile()
    return nc


_NC_CACHE = None


def _get_nc():
    global _NC_CACHE
    if _NC_CACHE is None:
        _NC_CACHE = _build()
    return _NC_CACHE


def _f8(x):
    return np.asarray(x, np.float32).astype(NPF8)


def _host_tables(position_ids: np.ndarray):
    pos = np.asarray(position_ids, np.float32)
    inv_freq = (1.0 / (THETA ** (np.arange(0, D, 2, dtype=np.float32) / D)))
    ang = pos[:, None] * inv_freq[None, :]          # [T, 64] f32
    dq = np.float32(1.0 / (S_H * S_W))              # QKV dequant folded in
    cos = (np.cos(ang).T * dq).astype(np.float32)   # [64, T]
    sin = (np.sin(ang).T * dq).astype(np.float32)
    cosT = np.concatenate([cos, cos], axis=0).astype(NPBF)   # [128, T]
    sinT = np.concatenate([-sin, sin], axis=0).astype(NPBF)
    return cosT, sinT


def _host_mask():
    r = np.arange(128)[:, None]
    c = np.arange(128)[None, :]
    return _f8((c - r >= 0).astype(np.float32))     # [128, 128] triangle


def _pack_w(w_local_scaled: np.ndarray):
    """[2048, 512] (already scaled) -> hi/res packed [128, 8, 2, 4, 128]."""
    hi = _f8(w_local_scaled)
    re = _f8(w_local_scaled - hi.astype(np.float32))

    def pack(a):
        # row r = j*256 + i*128 + p ; col = n*128 + c
        return np.ascontiguousarray(
            a.reshape(NPAIR, 2, 128, 4, 128).transpose(2, 0, 1, 3, 4))
    return pack(hi), pack(re)


def kernel(hidden_states, position_ids, Wqkv, Wo):
    hidden_states = np.asarray(hidden_states, np.float32)
    Wqkv = np.asarray(Wqkv, np.float32)
    Wo = np.asarray(Wo, np.float32)

    nc = _get_nc()

    hT_s = hidden_states.T * np.float32(S_H)
    h_hi = _f8(hT_s)
    h_re = _f8(hT_s - h_hi.astype(np.float32))
    cosT, sinT = _host_tables(position_ids)
    mask = _host_mask()
    ones = np.full((128, 2, 128), ALPHA, np.float32).astype(NPF8)

    wq = Wqkv[:, : H * D]
    wk = Wqkv[:, H * D: (H + KV) * D]
    wv = Wqkv[:, (H + KV) * D:]

    in_maps = []
    for c in range(N_CORES):
        kvh = (c * QH) // (H // KV)
        w_local = np.concatenate(
            [wq[:, (c * QH) * D: (c * QH + 1) * D],
             wq[:, (c * QH + 1) * D: (c * QH + 2) * D],
             wk[:, kvh * D: (kvh + 1) * D],
             wv[:, kvh * D: (kvh + 1) * D]], axis=1) * np.float32(S_W)
        w_hi, w_re = _pack_w(w_local)   # [128, 8, 2, 4, 128]
        w_split = {
            "w_hiA": np.ascontiguousarray(w_hi[:, :, :, 0:2, :]),
            "w_hiB": np.ascontiguousarray(w_hi[:, :, :, 2:4, :]),
            "w_reA": np.ascontiguousarray(w_re[:, :, :, 0:2, :]),
            "w_reB": np.ascontiguousarray(w_re[:, :, :, 2:4, :]),
        }
        wo_local = Wo[c * QH * D: (c + 1) * QH * D, :] * np.float32(S_WO)
        wo_hi = _f8(wo_local)
        wo_re = _f8(wo_local - wo_hi.astype(np.float32))
        # [2*128, HID] -> [128, 2, HID] (slot = head)
        wo_hi = np.ascontiguousarray(wo_hi.reshape(2, 128, HID).transpose(1, 0, 2))
        wo_re = np.ascontiguousarray(wo_re.reshape(2, 128, HID).transpose(1, 0, 2))
        in_maps.append({
            "h_hi": h_hi, "h_re": h_re,
            **w_split,
            "wo_hi": wo_hi, "wo_re": wo_re,
            "cosT": cosT, "sinT": sinT, "masks": mask,
            "ones": ones,
        })

    res = bass_utils.run_bass_kernel_spmd(nc, in_maps,
                                          core_ids=list(range(N_CORES)))
    parts = np.stack([res.results[c]["out_partial"].astype(np.float32)
                      for c in range(N_CORES)], 0)
    return parts.sum(axis=0, dtype=np.float32) * np.float32(1.0 / S_OUT)


# revision 64
# speedup vs baseline: 1.3955x; 1.0065x over previous
"""Bass/Trainium2 kernel for BailingAttention (GQA prefill, causal, RoPE).

Sharding: tensor-parallel over heads across 8 NeuronCores. Each core computes
2 query heads + its group's shared KV head end-to-end and writes a partial
[T, HID] output (bf16); the host sums the 8 partials (row-parallel
all-reduce) and applies the global dequant scale.

Numerics (rel tolerance 2e-2; this lands ~1e-2):
  - QKV and output projections are 3-term split-fp8 DoubleRow matmuls:
    x ~ hi + res, both e4m3 in SHARED scale units, so hi@hi + hi@res +
    res@hi accumulates in one PSUM group at ~0.1% error and 3/4 the fp32r
    PE cost (DoubleRow = 0.5 cyc/row over a 2x128 contraction).
  - Scores stay fp32r (the exp is error-sensitive).
  - exp outputs (e) are e4m3; PV and the softmax denominator run DoubleRow
    over adjacent key-tile PAIRS (e pair tiles [128,2,512]); v is hi+res.
  - All scales are powers of two, folded into host tables (cos/sin carry
    the QKV dequant, the denominator 'ones' stationary carries the ctx
    scale, the final dequant happens host-side during the partial sum).

Schedule: one software-pipelined emission stream. Per 512-token block b:
pass A (q heads) QKV matmuls interleaved with attention(b-1) leftovers and
out-proj(b-1); RoPE(q); pass B (k/v) interleaved with attention(b) head 0's
non-diagonal work. One attention head is in flight at a time so PSUM fits:
QKV ring 2 banks + score-pair ring 4 + ctx 1 + den 1 = 8.

Engine discipline (each engine's FIFO order gates its consumers):
  ACT  = exps only, plus issue-only DMAs (xsw swaps, v DMA-transpose).
  DVE  = psum evicts (q0/k/v), RoPE m1+add, softmax recip/c32/ctx_hi,
         half the out-proj evicts.
  Pool = q1 evict, RoPE t2, causal tri-masks+memsets (shrunk to [128,128]),
         ctx_re, v quantize.
  SP   = all dependency-free input DMAs + output DMAs (emitted late).
Diagonal score pairs are exact-causal: the moving range shrinks to >=256
columns, exp shrinks with it, masks act on [128,128] triangles only.
"""

import numpy as np
import ml_dtypes

import concourse.bass as bass
import concourse.mybir as mybir
import concourse.tile as tile
from concourse import bacc, bass_utils
from concourse.bass import ts

F32 = mybir.dt.float32
F32R = mybir.dt.float32r
F8 = mybir.dt.float8e4
BF16 = mybir.dt.bfloat16
AF = mybir.ActivationFunctionType
OP = mybir.AluOpType
DR = mybir.MatmulPerfMode.DoubleRow
NPF8 = ml_dtypes.float8_e4m3
NPBF = ml_dtypes.bfloat16

H, KV, D, HID, T = 16, 4, 128, 2048, 2048
THETA = 10000.0
N_CORES = 8
QH = H // N_CORES            # query heads per core = 2
TB = 512                     # token block (matmul moving N)
NTB = T // TB                # 4
NPAIR = HID // 256           # 8 contraction pairs for QKV
SCALE = float(D) ** -0.5

S_H = 32.0                   # hidden quant scale
S_W = 1024.0                 # Wqkv quant scale
S_V = 4.0                    # on-device v scale (v tiles = 4*v_true)
S_ADJ = S_V / (S_H * S_W)    # v psum -> scaled-v evict factor
ALPHA = 0.125                # ones value: ctx_hi = (S_V/ALPHA)*ctx = 32*ctx
S_WO = 1024.0                # Wo quant scale
S_OUT = (S_V / ALPHA) * S_WO  # host-side dequant of the partial outputs


def _riffle(a, b):
    """Proportionally merge two unit lists into one."""
    if not a or not b:
        return a + b
    out = []
    r = len(a) / len(b)
    bi = 0
    acc = 0.0
    for u in a:
        out.append(u)
        acc += 1.0
        while bi < len(b) and acc >= r:
            out.append(b[bi])
            bi += 1
            acc -= r
    out.extend(b[bi:])
    return out


def _interleave(main, filler):
    """Emit `main` and `filler` unit lists proportionally merged."""
    if not filler or not main:
        for u in main + filler:
            u()
        return
    r = len(main) / len(filler)
    fi = 0
    acc = 0.0
    for u in main:
        u()
        acc += 1.0
        while fi < len(filler) and acc >= r:
            filler[fi]()
            fi += 1
            acc -= r
    while fi < len(filler):
        filler[fi]()
        fi += 1


def _build():
    nc = bacc.Bacc("TRN2", target_bir_lowering=False, debug=False,
                   num_devices=N_CORES)

    h_hi_d = nc.dram_tensor("h_hi", [HID, T], F8, kind="ExternalInput").ap()
    h_re_d = nc.dram_tensor("h_re", [HID, T], F8, kind="ExternalInput").ap()
    # w split by output-column pair: A = n in (0,1) [q heads], B = n in (2,3)
    w_d = {}
    for hv in ("hi", "re"):
        for ab in ("A", "B"):
            w_d[(hv, ab)] = nc.dram_tensor(
                f"w_{hv}{ab}", [128, NPAIR, 2, 2, 128], F8,
                kind="ExternalInput").ap()
    wo_hi_d = nc.dram_tensor("wo_hi", [128, 2, HID], F8,
                             kind="ExternalInput").ap()
    wo_re_d = nc.dram_tensor("wo_re", [128, 2, HID], F8,
                             kind="ExternalInput").ap()
    cos_d = nc.dram_tensor("cosT", [128, T], BF16, kind="ExternalInput").ap()
    sin_d = nc.dram_tensor("sinT", [128, T], BF16, kind="ExternalInput").ap()
    mask_d = nc.dram_tensor("masks", [128, 128], F8, kind="ExternalInput").ap()
    ones_d = nc.dram_tensor("ones", [128, 2, 128], F8, kind="ExternalInput").ap()
    out_d = nc.dram_tensor("out_partial", [T, HID], BF16, kind="ExternalOutput").ap()

    with tile.TileContext(nc) as tc:
        with tc.tile_pool(name="const", bufs=1) as cpool, \
             tc.tile_pool(name="acts", bufs=1) as apool, \
             tc.tile_pool(name="hstream", bufs=2) as hpool, \
             tc.tile_pool(name="rope", bufs=2) as tpool, \
             tc.tile_pool(name="exp", bufs=12) as epool, \
             tc.tile_pool(name="ctmp", bufs=3) as t2pool, \
             tc.tile_pool(name="outsb", bufs=3) as opool, \
             tc.tile_pool(name="qkvps", bufs=2, space="PSUM") as qkv_ps, \
             tc.tile_pool(name="sps", bufs=2, space="PSUM") as spool, \
             tc.tile_pool(name="cps", bufs=1, space="PSUM") as cpsp, \
             tc.tile_pool(name="dps", bufs=1, space="PSUM") as dpsp:

            w_sb = {k: cpool.tile([128, NPAIR, 2, 2, 128], F8,
                                  name=f"w_{k[0]}{k[1]}", tag=f"w_{k[0]}{k[1]}")
                    for k in w_d}
            wo_hi = cpool.tile([128, 2, HID], F8)
            wo_re = cpool.tile([128, 2, HID], F8)
            cos_sb = cpool.tile([128, T], BF16)
            sin_sb = cpool.tile([128, T], BF16)
            mask_sb = cpool.tile([128, 128], F8)
            ones_sb = cpool.tile([128, 2, 128], F8)

            qrT = [[apool.tile([128, TB], F32R, name=f"q{i}b{b}", tag=f"q{i}b{b}")
                    for b in range(NTB)] for i in range(QH)]
            krT = [apool.tile([128, TB], F32R, name=f"kb{b}", tag=f"kb{b}")
                   for b in range(NTB)]
            vbf = [apool.tile([128, 4, 128], BF16, name=f"vbf{b}", tag=f"vbf{b}")
                   for b in range(NTB)]
            v_hi = [apool.tile([128, 2, 2, 128], F8, name=f"vhb{b}", tag=f"vhb{b}")
                    for b in range(NTB)]
            v_re = [apool.tile([128, 2, 2, 128], F8, name=f"vrb{b}", tag=f"vrb{b}")
                    for b in range(NTB)]
            ctx_hi = [apool.tile([128, 2, TB], F8, name=f"chb{b}", tag=f"chb{b}")
                      for b in range(NTB)]
            ctx_re = [apool.tile([128, 2, TB], F8, name=f"crb{b}", tag=f"crb{b}")
                      for b in range(NTB)]

            h_hi_v = h_hi_d.rearrange("(j i p) t -> p j i t", i=2, p=128)
            h_re_v = h_re_d.rearrange("(j i p) t -> p j i t", i=2, p=128)

            h_tiles = {}
            qkv_state = {}
            att_state = {}
            rr = {"osb": 0}

            def u_dma_h(b, hv):
                """Load one h stream (hi or re) for block b as two half-slabs
                on the SP queue."""
                def run():
                    src = h_hi_v if hv == "hi" else h_re_v
                    for ab, j0 in (("A", 0), ("B", 4)):
                        t = hpool.tile([128, 4, 2, TB], F8, tag=f"h{hv}{ab}")
                        h_tiles[(b, hv, ab)] = t
                        nc.sync.dma_start(t[:], src[:, j0:j0 + 4, :, ts(b, TB)])
                return run

            def u_dma_w(hv, ab):
                def run():
                    nc.sync.dma_start(w_sb[(hv, ab)][:], w_d[(hv, ab)])
                return run

            def u_dma_tables(b):
                def run():
                    nc.sync.dma_start(cos_sb[:, ts(b, TB)], cos_d[:, ts(b, TB)])
                    nc.sync.dma_start(sin_sb[:, ts(b, TB)], sin_d[:, ts(b, TB)])
                    if b == 0:
                        nc.sync.dma_start(ones_sb[:], ones_d)
                        nc.sync.dma_start(mask_sb[:], mask_d)
                return run

            def u_dma_const1():
                def run():
                    nc.sync.dma_start(wo_hi[:], wo_hi_d)
                    nc.sync.dma_start(wo_re[:], wo_re_d)
                return run

            # ---------------- QKV stream ------------------------------------
            def u_qkv_mm(b, n, stream, jh):
                """One unit = 4 DoubleRow matmuls (j = jh*4 .. jh*4+3)."""
                def run():
                    ab = "A" if jh == 0 else "B"
                    wab = "A" if n < 2 else "B"
                    if stream == 0 and jh == 0:
                        qkv_state[(b, n)] = qkv_ps.tile(
                            [128, TB], F32, name=f"qkv{n}", tag="qkv")
                    ps = qkv_state[(b, n)]
                    wv, hv = [("hi", "hi"), ("hi", "re"), ("re", "hi")][stream]
                    wt = w_sb[(wv, wab)]
                    ht = h_tiles[(b, hv, ab)]
                    for jj in range(4):
                        j = jh * 4 + jj
                        nc.tensor.matmul(
                            ps[:], wt[:, j, :, n % 2, :], ht[:, jj],
                            perf_mode=DR,
                            start=(stream == 0 and j == 0),
                            stop=(stream == 2 and j == NPAIR - 1))
                return run

            def u_evict_rope(b, n):
                """Evict qkv psum n (q0/q1/k) and run its RoPE chain."""
                def run():
                    ps = qkv_state.pop((b, n))
                    x_sb = tpool.tile([128, TB], F32, tag=f"x{n}")
                    if n == 1:
                        nc.scalar.copy(x_sb[:], ps[:])
                    else:
                        nc.vector.tensor_copy(x_sb[:], ps[:])
                    dst = qrT[n][b] if n < QH else krT[b]
                    xsw = tpool.tile([128, TB], F32, tag=f"xsw{n}")
                    nc.sync.dma_start(xsw[0:64, :], x_sb[64:128, :])
                    nc.sync.dma_start(xsw[64:128, :], x_sb[0:64, :])
                    t2 = tpool.tile([128, TB], F32, tag=f"t2{n}")
                    m1 = tpool.tile([128, TB], F32, tag=f"m1{n}")
                    nc.vector.tensor_tensor(out=m1[:], in0=x_sb[:],
                                            in1=cos_sb[:, ts(b, TB)], op=OP.mult)
                    nc.gpsimd.tensor_tensor(out=t2[:], in0=xsw[:],
                                            in1=sin_sb[:, ts(b, TB)], op=OP.mult)
                    nc.vector.tensor_tensor(out=dst[:], in0=m1[:], in1=t2[:],
                                            op=OP.add)
                return run

            def u_evict_v(b):
                def run():
                    ps = qkv_state.pop((b, 3))
                    x_sb = tpool.tile([128, TB], BF16, tag="xv")
                    nc.vector.tensor_scalar_mul(x_sb[:], ps[:], S_ADJ)
                    qkv_state[("vT", b)] = x_sb
                return run

            def u_vtrans(b, jj):
                def run():
                    vT_sb = qkv_state[("vT", b)]
                    nc.sync.dma_start_transpose(vbf[b][:, jj, :],
                                                  vT_sb[:, ts(jj, 128)])
                return run

            def u_vquant(b, jj):
                def run():
                    p, s = jj // 2, jj % 2
                    nc.vector.tensor_copy(v_hi[b][:, p, s, :], vbf[b][:, jj, :])
                    nc.vector.tensor_tensor(out=v_re[b][:, p, s, :],
                                            in0=vbf[b][:, jj, :],
                                            in1=v_hi[b][:, p, s, :],
                                            op=OP.subtract)
                return run

            # ---------------- attention stream ------------------------------
            def qlo_of(bb, p):
                """Exact-causal moving-range start for pair p of block bb
                (clamped so fp32r keeps >=256 moving columns)."""
                k0 = 2 * p - 4 * bb          # first local key tile of the pair
                if k0 < 0:
                    return 0
                return min(128 * k0, TB - 256)

            def u_score_pair(bb, qh, p):
                """Two fp32r score matmuls + one paired exp (+ diag masks)."""
                def run():
                    st = att_state.setdefault((bb, qh), {})
                    s_pair = spool.tile([128, 2, TB], F32, name="s_pair",
                                        tag="sps")
                    e_pair = epool.tile([128, 2, TB], F8, name="e_pair",
                                        tag="exp")
                    st[("e", p)] = e_pair
                    q0 = qlo_of(bb, p)
                    for s in range(2):
                        kt = 2 * p + s
                        nc.tensor.matmul(s_pair[:, s, q0:TB],
                                         krT[kt // 4][:, ts(kt % 4, 128)],
                                         qrT[qh][bb][:, q0:TB],
                                         start=True, stop=True)
                    nc.scalar.activation(e_pair[:, :, q0:TB],
                                         s_pair[:, :, q0:TB], AF.Exp,
                                         scale=SCALE)
                    if 2 * p + 1 >= 4 * bb:   # diagonal pair: causal masks
                        for s in range(2):
                            kt_l = 2 * p + s - 4 * bb
                            c0 = 128 * kt_l
                            nc.gpsimd.tensor_tensor(
                                out=e_pair[:, s, c0:c0 + 128],
                                in0=e_pair[:, s, c0:c0 + 128],
                                in1=mask_sb[:], op=OP.mult)
                            if s == 1 and c0 > q0:
                                nc.gpsimd.memset(e_pair[:, 1, q0:c0], 0.0)
                return run

            def u_consume(bb, qh, p, npair):
                def run():
                    st = att_state[(bb, qh)]
                    if p == 0:
                        st["ctx"] = cpsp.tile([128, TB], F32, name="ctx_ps")
                        st["den"] = dpsp.tile([128, TB], F32, name="den_ps")
                    e_pair = st.pop(("e", p))
                    first = (p == 0)
                    last = (p == npair - 1)
                    q0 = qlo_of(bb, p)
                    vb, vp = p // 2, p % 2
                    nc.tensor.matmul(st["ctx"][:, q0:TB], v_hi[vb][:, vp, :, :],
                                     e_pair[:, :, q0:TB], perf_mode=DR,
                                     start=first, stop=False)
                    nc.tensor.matmul(st["ctx"][:, q0:TB], v_re[vb][:, vp, :, :],
                                     e_pair[:, :, q0:TB], perf_mode=DR,
                                     start=False, stop=last)
                    nc.tensor.matmul(st["den"][:, q0:TB], ones_sb[:],
                                     e_pair[:, :, q0:TB], perf_mode=DR,
                                     start=first, stop=last)
                return run

            def u_ctx1(bb, qh):
                def run():
                    st = att_state[(bb, qh)]
                    recip = t2pool.tile([128, TB], F32, tag="recip")
                    c32 = t2pool.tile([128, TB], F32, tag="c32")
                    nc.vector.reciprocal(recip[:], st["den"][:])
                    nc.vector.tensor_tensor(out=c32[:], in0=st["ctx"][:],
                                            in1=recip[:], op=OP.mult)
                    st["c32"] = c32
                return run

            def u_ctx2(bb, qh):
                def run():
                    st = att_state.pop((bb, qh))
                    c32 = st["c32"]
                    nc.vector.tensor_copy(ctx_hi[bb][:, qh, :], c32[:])
                    nc.gpsimd.tensor_tensor(out=ctx_re[bb][:, qh, :],
                                            in0=c32[:],
                                            in1=ctx_hi[bb][:, qh, :],
                                            op=OP.subtract)
                return run

            def att_units(bb, qh, part):
                """nd: scores+consumes for non-diagonal pairs; nd_s/nd_c:
                scores-only / consumes-only variants (e pairs buffered in
                epool between them); d: diagonal pairs + softmax chain."""
                npair = 2 * (bb + 1)
                if part in ("nd", "nd_s", "nd_c"):
                    pairs = range(0, 2 * bb)
                else:
                    pairs = range(2 * bb, npair)
                units = []
                pend = []
                for p in pairs:
                    if part != "nd_c":
                        units.append(u_score_pair(bb, qh, p))
                    if part != "nd_s":
                        if part in ("nd", "d"):
                            pend.append(u_consume(bb, qh, p, npair))
                            if len(pend) > 2:
                                units.append(pend.pop(0))
                        else:
                            units.append(u_consume(bb, qh, p, npair))
                units.extend(pend)
                if part == "d":
                    units.append(u_ctx1(bb, qh))
                    units.append(u_ctx2(bb, qh))
                return units

            # ---------------- output-projection stream ----------------------
            def out_units(bb):
                units = []
                st = {}

                def u_alloc(tt, st=st):
                    def run():
                        st[tt] = opool.tile([128, 2, 2, TB], BF16, name="o_sb")
                    return run

                def u_proj(tt, half, bb=bb, st=st):
                    def run():
                        ps = spool.tile([128, 2, TB], F32, name="ps_o",
                                        tag="sps")
                        ch = ctx_hi[bb][:, :, ts(tt % 4, 128)]
                        cr = ctx_re[bb][:, :, ts(tt % 4, 128)]
                        for s in range(2):
                            n = 2 * half + s
                            nc.tensor.matmul(ps[:, s, :], ch,
                                             wo_hi[:, :, ts(n, 512)],
                                             perf_mode=DR, start=True, stop=False)
                            nc.tensor.matmul(ps[:, s, :], ch,
                                             wo_re[:, :, ts(n, 512)],
                                             perf_mode=DR, start=False, stop=False)
                            nc.tensor.matmul(ps[:, s, :], cr,
                                             wo_hi[:, :, ts(n, 512)],
                                             perf_mode=DR, start=False, stop=True)
                        rr["osb"] += 1
                        if bb != NTB - 1 and rr["osb"] % 2:
                            nc.vector.tensor_copy(st[tt][:, half, :, :], ps[:])
                        else:
                            nc.scalar.copy(st[tt][:, half, :, :], ps[:])
                    return run

                def u_odma(tt, half, st=st):
                    def run():
                        o = st[tt] if half == 0 else st.pop(tt)
                        nc.sync.dma_start(
                            out_d[ts(tt, 128), ts(half, 1024)], o[:, half])
                    return run

                # emit the DMA for tile tt after the next tile's first proj
                # so the SP queue never head-of-line blocks on the evict.
                pend = []
                for tt in range(4 * bb, 4 * bb + 4):
                    units.append(u_alloc(tt))
                    units.append(u_proj(tt, 0))
                    if pend:
                        units.append(pend.pop(0))
                    units.append(u_proj(tt, 1))
                    pend.append(u_odma(tt, 0))
                    pend.append(u_odma(tt, 1))
                units += pend
                return units

            # ---------------- merged emission --------------------------------
            def qkv_pass(b, ns):
                units = []
                for stream in range(3):
                    for n in ns:
                        for jh in range(2):
                            units.append(u_qkv_mm(b, n, stream, jh))
                return units

            for b in range(NTB):
                # ---- pass A (q heads) ----
                mainA = []
                if b == 0:
                    mainA.append(u_dma_w("hi", "A"))
                    mainA.append(u_dma_h(0, "hi"))
                    mainA.append(u_dma_h(0, "re"))
                    mainA.append(u_dma_w("re", "A"))
                mainA += qkv_pass(b, (0, 1))
                if b == 0:
                    mainA.insert(7, u_dma_w("hi", "B"))
                    mainA.insert(8, u_dma_w("re", "B"))
                else:
                    mainA.insert(6, u_dma_h(b + 1, "hi") if b + 1 < NTB
                                 else (lambda: None))
                mainA.append(u_dma_tables(b))
                if b + 1 < NTB:
                    if b == 0:
                        mainA.append(u_dma_h(b + 1, "hi"))
                    mainA.append(u_dma_h(b + 1, "re"))
                fillerA = []
                if b > 0:
                    fillerA += [u_vtrans(b - 1, jj) for jj in range(4)]
                    fillerA += [u_vquant(b - 1, jj) for jj in range(4)]
                    fillerA += att_units(b - 1, 0, "d")
                    fillerA += att_units(b - 1, 1, "nd_c")
                    fillerA += att_units(b - 1, 1, "d")
                _interleave(mainA, fillerA)
                # ---- RoPE for q heads ----
                u_evict_rope(b, 0)()
                u_evict_rope(b, 1)()
                # ---- pass B (k, v) ----
                mainB = qkv_pass(b, (2, 3))
                if b == 0:
                    mainB.append(u_dma_const1())
                att_b = att_units(b, 0, "nd") + att_units(b, 1, "nd_s")
                fillerB = _riffle(out_units(b - 1) if b > 0 else [], att_b)
                _interleave(mainB, fillerB)
                u_evict_rope(b, 2)()
                u_evict_v(b)()

            # ---- tail: attention(last) + out(last) ----
            bl = NTB - 1
            tail = [u_vtrans(bl, jj) for jj in range(4)]
            tail += [u_vquant(bl, jj) for jj in range(4)]
            tail += att_units(bl, 0, "d")
            tail += att_units(bl, 1, "nd_c")
            tail += att_units(bl, 1, "d")
            tail += out_units(bl)
            for u in tail:
                u()

    nc.compile()
    return nc


_NC_CACHE = None


def _get_nc():
    global _NC_CACHE
    if _NC_CACHE is None:
        _NC_CACHE = _build()
    return _NC_CACHE


def _f8(x):
    return np.asarray(x, np.float32).astype(NPF8)


def _host_tables(position_ids: np.ndarray):
    pos = np.asarray(position_ids, np.float32)
    inv_freq = (1.0 / (THETA ** (np.arange(0, D, 2, dtype=np.float32) / D)))
    ang = pos[:, None] * inv_freq[None, :]          # [T, 64] f32
    dq = np.float32(1.0 / (S_H * S_W))              # QKV dequant folded in
    cos = (np.cos(ang).T * dq).astype(np.float32)   # [64, T]
    sin = (np.sin(ang).T * dq).astype(np.float32)
    cosT = np.concatenate([cos, cos], axis=0).astype(NPBF)   # [128, T]
    sinT = np.concatenate([-sin, sin], axis=0).astype(NPBF)
    return cosT, sinT


def _host_mask():
    r = np.arange(128)[:, None]
    c = np.arange(128)[None, :]
    return _f8((c - r >= 0).astype(np.float32))     # [128, 128] triangle


def _pack_w(w_local_scaled: np.ndarray):
    """[2048, 512] (already scaled) -> hi/res packed [128, 8, 2, 4, 128]."""
    hi = _f8(w_local_scaled)
    re = _f8(w_local_scaled - hi.astype(np.float32))

    def pack(a):
        # row r = j*256 + i*128 + p ; col = n*128 + c
        return np.ascontiguousarray(
            a.reshape(NPAIR, 2, 128, 4, 128).transpose(2, 0, 1, 3, 4))
    return pack(hi), pack(re)


def kernel(hidden_states, position_ids, Wqkv, Wo):
    hidden_states = np.asarray(hidden_states, np.float32)
    Wqkv = np.asarray(Wqkv, np.float32)
    Wo = np.asarray(Wo, np.float32)

    nc = _get_nc()

    hT_s = hidden_states.T * np.float32(S_H)
    h_hi = _f8(hT_s)
    h_re = _f8(hT_s - h_hi.astype(np.float32))
    cosT, sinT = _host_tables(position_ids)
    mask = _host_mask()
    ones = np.full((128, 2, 128), ALPHA, np.float32).astype(NPF8)

    wq = Wqkv[:, : H * D]
    wk = Wqkv[:, H * D: (H + KV) * D]
    wv = Wqkv[:, (H + KV) * D:]

    in_maps = []
    for c in range(N_CORES):
        kvh = (c * QH) // (H // KV)
        w_local = np.concatenate(
            [wq[:, (c * QH) * D: (c * QH + 1) * D],
             wq[:, (c * QH + 1) * D: (c * QH + 2) * D],
             wk[:, kvh * D: (kvh + 1) * D],
             wv[:, kvh * D: (kvh + 1) * D]], axis=1) * np.float32(S_W)
        w_hi, w_re = _pack_w(w_local)   # [128, 8, 2, 4, 128]
        w_split = {
            "w_hiA": np.ascontiguousarray(w_hi[:, :, :, 0:2, :]),
            "w_hiB": np.ascontiguousarray(w_hi[:, :, :, 2:4, :]),
            "w_reA": np.ascontiguousarray(w_re[:, :, :, 0:2, :]),
            "w_reB": np.ascontiguousarray(w_re[:, :, :, 2:4, :]),
        }
        wo_local = Wo[c * QH * D: (c + 1) * QH * D, :] * np.float32(S_WO)
        wo_hi = _f8(wo_local)
        wo_re = _f8(wo_local - wo_hi.astype(np.float32))
        # [2*128, HID] -> [128, 2, HID] (slot = head)
        wo_hi = np.ascontiguousarray(wo_hi.reshape(2, 128, HID).transpose(1, 0, 2))
        wo_re = np.ascontiguousarray(wo_re.reshape(2, 128, HID).transpose(1, 0, 2))
        in_maps.append({
            "h_hi": h_hi, "h_re": h_re,
            **w_split,
            "wo_hi": wo_hi, "wo_re": wo_re,
            "cosT": cosT, "sinT": sinT, "masks": mask,
            "ones": ones,
        })

    res = bass_utils.run_bass_kernel_spmd(nc, in_maps,
                                          core_ids=list(range(N_CORES)))
    parts = np.stack([res.results[c]["out_partial"].astype(np.float32)
                      for c in range(N_CORES)], 0)
    return parts.sum(axis=0, dtype=np.float32) * np.float32(1.0 / S_OUT)
